# revision 12
# baseline (speedup 1.0000x reference)
"""Trainium2 Bass kernel for nn_KumaMask: z = clip(kuma_rsample(softplus(x@Wa+ba),
softplus(x@Wb+bb)) * 1.2 - 0.1, 0, 1) with fixed RNG noise (key 42).

Strategy (pure data parallel over 8 cores, 16384 tokens/core):
  - The uniform noise u depends only on a fixed key/shape -> precompute log(u)
    on host (f64 log, rounded to f32) and ship as a constant table.
  - Per 128-token tile: PE transposes the 4 [128,128] quadrants of x
    (PSUM), DVE/ACT copy them back to SBUF, then PE contracts with the
    stationary [128,2] (Wa|Wb) chunks accumulating over the 4 d-chunks into a
    [2, 128] PSUM slab. The [2, 16384] result is PE-de-transposed into a
    [128, 256] token-major PSUM layout for the cheap transcendental tail.
  - Tail: a=ln(1+exp(pa+ba)), b likewise, t=exp(log(u)/b),
    z = min(relu(1.2*exp(log1p(-t)/a) - 0.1), 1).
"""

import numpy as np

B, T, D = 32, 4096, 512
N_CORES = 8
TOK_PER_CORE = B * T // N_CORES  # 16384
NTILES = TOK_PER_CORE // 128     # 128
NQ = D // 128                    # 4 contraction chunks

_PROGRAM_CACHE = {}


def _compute_lu():
    """log(u) for the reference's fixed-key uniform draw, best f32 rounding."""
    import jax
    import jax.numpy as jnp

    cpu = jax.devices("cpu")[0]
    with jax.default_device(cpu):
        u = jax.random.uniform(
            jax.random.key(42), (B, T, 1), dtype=jnp.float32,
            minval=float(np.finfo(np.float32).tiny), maxval=1.0,
        )
        u = np.asarray(u, dtype=np.float32).reshape(-1)
    return np.log(u.astype(np.float64)).astype(np.float32)


def _build_program(ba: float, bb: float):
    from contextlib import ExitStack

    import concourse.bacc as bacc
    import concourse.tile as tile
    from concourse import mybir

    f32 = mybir.dt.float32
    AFT = mybir.ActivationFunctionType

    nc = bacc.Bacc("TRN2", target_bir_lowering=False, debug=False)

    xs = nc.dram_tensor("xs", [TOK_PER_CORE, D], f32, kind="ExternalInput").ap()
    wab = nc.dram_tensor("wab", [128, 2 * NQ], f32, kind="ExternalInput").ap()
    lu = nc.dram_tensor("lu", [128, NTILES], f32, kind="ExternalInput").ap()
    ident = nc.dram_tensor("ident", [128, 128], f32, kind="ExternalInput").ap()
    zs = nc.dram_tensor("zs", [128, NTILES], f32, kind="ExternalOutput").ap()

    xs_t = xs.rearrange("(j p) d -> j p d", p=128)

    with tile.TileContext(nc) as tc, ExitStack() as ctx:
        const = ctx.enter_context(tc.tile_pool(name="const", bufs=1))
        xin = ctx.enter_context(tc.tile_pool(name="xin", bufs=6))
        xtp = ctx.enter_context(tc.tile_pool(name="xtp", bufs=4))
        pxta = ctx.enter_context(tc.tile_pool(name="pxta", bufs=2, space="PSUM"))
        pxtb = ctx.enter_context(tc.tile_pool(name="pxtb", bufs=2, space="PSUM"))
        warmp = ctx.enter_context(tc.tile_pool(name="warmp", bufs=1, space="PSUM"))
        pyp = ctx.enter_context(tc.tile_pool(name="pyp", bufs=2, space="PSUM"))
        ysb = ctx.enter_context(tc.tile_pool(name="ysb", bufs=1))
        pabp = ctx.enter_context(tc.tile_pool(name="pabp", bufs=1, space="PSUM"))
        post = ctx.enter_context(tc.tile_pool(name="post", bufs=1))

        w_sb = const.tile([128, 2 * NQ], f32)
        w_dma = nc.sync.dma_start(out=w_sb, in_=wab)
        i_sb = const.tile([128, 128], f32)
        i_dma = nc.sync.dma_start(out=i_sb, in_=ident)
        lu_sb = const.tile([128, NTILES], f32)
        nc.sync.dma_start(out=lu_sb, in_=lu)

        y_sb = ysb.tile([2, TOK_PER_CORE], f32)
        pab = pabp.tile([128, 2 * NTILES], f32)

        bias_a = const.tile([128, 1], f32)
        nc.vector.memset(bias_a, ba)
        bias_b = const.tile([128, 1], f32)
        nc.vector.memset(bias_b, bb)
        bias_r = const.tile([128, 1], f32)
        nc.vector.memset(bias_r, -0.1)

        # Warmup: absorb the const-DMA waits into PE's vector clock one at a
        # time (walrus allows only one sync wait per Matmult). Scratch output.
        wt = warmp.tile([2, 4], f32)
        nc.tensor.transpose(wt[:, 0:2], i_sb[0:2, 0:2], i_sb[0:2, 0:2])
        nc.tensor.matmul(wt[:, 2:4], lhsT=w_sb[:, 0:2], rhs=i_sb[:, 0:2],
                         start=True, stop=True)

        copy_insts = [None] * NTILES
        ycopy_insts = [None] * (NTILES // 4)
        py = None
        for j in range(NTILES):
            g, t4 = divmod(j, 4)
            x_t = xin.tile([128, D], f32)
            x_dma = nc.sync.dma_start(out=x_t, in_=xs_t[j])

            px = (pxta if j % 2 == 0 else pxtb).tile([128, D], f32, name="px")
            for q in range(NQ):
                nc.tensor.transpose(
                    px[:, q * 128:(q + 1) * 128],
                    x_t[:, q * 128:(q + 1) * 128],
                    i_sb,
                )
            xt = xtp.tile([128, D], f32)
            if j % 2 == 0:
                copy_insts[j] = nc.vector.tensor_copy(xt, px)
            else:
                copy_insts[j] = nc.scalar.copy(xt, px)

            if t4 == 0:
                py = pyp.tile([2, 512], f32)
            for q in range(NQ):
                nc.tensor.matmul(
                    py[:, t4 * 128:(t4 + 1) * 128],
                    lhsT=w_sb[:, 2 * q:2 * q + 2],
                    rhs=xt[:, q * 128:(q + 1) * 128],
                    start=(q == 0),
                    stop=(q == NQ - 1),
                )
            if t4 == 3:
                ycopy_insts[g] = nc.vector.tensor_copy(
                    y_sb[:, g * 512:(g + 1) * 512], py
                )

        for j in range(NTILES):
            nc.tensor.transpose(
                pab[:, 2 * j:2 * j + 2],
                y_sb[:, j * 128:(j + 1) * 128],
                i_sb[0:2, 0:2],
            )

        pab3 = pab[:, :].rearrange("p (j two) -> p j two", two=2)
        pa = pab3[:, :, 0]
        pb = pab3[:, :, 1]

        def ptile(nm):
            return post.tile([128, NTILES], f32, name=nm)

        ea = ptile("ea")
        nc.scalar.activation(ea, pa, AFT.Exp, bias=bias_a[:, :], scale=1.0)
        a = ptile("a")
        nc.scalar.activation(a, ea, AFT.Ln, bias=1.0, scale=1.0)
        ra = ptile("ra")
        nc.vector.reciprocal(ra, a)

        eb = ptile("eb")
        nc.scalar.activation(eb, pb, AFT.Exp, bias=bias_b[:, :], scale=1.0)
        b = ptile("b")
        nc.scalar.activation(b, eb, AFT.Ln, bias=1.0, scale=1.0)
        rb = ptile("rb")
        nc.vector.reciprocal(rb, b)

        m = ptile("m")
        nc.vector.tensor_mul(m, lu_sb, rb)
        t = ptile("t")
        nc.scalar.activation(t, m, AFT.Exp)
        s = ptile("s")
        nc.scalar.activation(s, t, AFT.Ln, bias=1.0, scale=-1.0)
        w2 = ptile("w2")
        nc.vector.tensor_mul(w2, s, ra)
        k = ptile("k")
        nc.scalar.activation(k, w2, AFT.Exp)
        z0 = ptile("z0")
        nc.scalar.activation(z0, k, AFT.Relu, bias=bias_r[:, :], scale=1.2)
        z = ptile("z")
        nc.vector.tensor_scalar_min(z, z0, 1.0)

        nc.sync.dma_start(out=zs, in_=z)

    nc.compile()
    return nc


def _pack_inputs(x, Wa, Wb):
    xs_all = np.ascontiguousarray(np.asarray(x, np.float32).reshape(-1, D))
    lu_all = _compute_lu()
    wab = np.empty((128, 2 * NQ), np.float32)
    wa = np.asarray(Wa, np.float32).reshape(D)
    wb = np.asarray(Wb, np.float32).reshape(D)
    for q in range(NQ):
        wab[:, 2 * q] = wa[q * 128:(q + 1) * 128]
        wab[:, 2 * q + 1] = wb[q * 128:(q + 1) * 128]
    identity = np.eye(128, dtype=np.float32)
    in_maps = []
    for c in range(N_CORES):
        sl = slice(c * TOK_PER_CORE, (c + 1) * TOK_PER_CORE)
        lus = np.ascontiguousarray(
            lu_all[sl].reshape(NTILES, 128).T
        )  # [p, j]: token 128j + p
        in_maps.append({
            "xs": xs_all[sl],
            "wab": wab,
            "lu": lus,
            "ident": identity,
        })
    return in_maps


def _unpack_output(results):
    z = np.empty((N_CORES, TOK_PER_CORE), np.float32)
    for c in range(N_CORES):
        zc = np.asarray(results[c]["zs"])  # [p, j]
        z[c] = zc.T.reshape(-1)            # token 128j + p
    return z.reshape(B, T, 1)


def kernel(x, Wa, ba, Wb, bb, _run_kwargs=None):
    from concourse.bass_utils import run_bass_kernel_spmd

    key = (float(np.asarray(ba).reshape(-1)[0]), float(np.asarray(bb).reshape(-1)[0]))
    if key not in _PROGRAM_CACHE:
        _PROGRAM_CACHE[key] = _build_program(*key)
    nc = _PROGRAM_CACHE[key]
    in_maps = _pack_inputs(x, Wa, Wb)
    res = run_bass_kernel_spmd(
        nc, in_maps, core_ids=list(range(N_CORES)), **(_run_kwargs or {})
    )
    out = _unpack_output(res.results)
    if _run_kwargs:
        kernel.last_result = res
    return out
